# revision 23
# baseline (speedup 1.0000x reference)
"""Multi-head cross-attention (B=2, N=1024, L=4096, D=1024, H=16) on 8 trn2
NeuronCores — bf16 pipeline (~257us, vs 350-395us fp32r baseline).

Sharding: batch x head-group data/tensor parallel. Core c handles batch
c//4 and heads 4*(c%4) .. 4*(c%4)+3 (weight columns sliced per head group,
Wo row-sliced; partial outputs summed on the host during unsharding).

Design notes:
  - all matmul operands bf16 (fp32 PSUM accumulate): fp32 moving operands
    stream at 2 cycles/col on the PE xbus, bf16 at 1 — halves matmul time
    and DMA traffic, and the DMA'd bf16 tiles feed matmuls directly (no
    fp32->fp32r DVE casts). rel_absmax ~5.6e-3 vs the fp32 reference.
  - padding mask applied as a per-key additive bias (-60) inside the exp
    activation (bias is a [128,1] per-partition AP), so V needs no keep
    premultiply; the augmented-V ones column provides the softmax
    denominator (row 64 of the attention-output PSUM accumulator).
  - q/k biases folded into the PSUM->SBUF copies (tensor_scalar_add with a
    per-partition bias vector); v bias added during the va build (DVE
    tensor_tensor add); no bias matmuls.
  - weights pre-chunked on the host to partition-major [128, chunk, F] so
    weight DMA reads contiguous 4KB lines; weights and half the x tiles
    ride the ACT hardware DMA queue, the rest the SP queue (2 queues in
    parallel, ~200GB/s each).
  - software pipeline: phase A interleaves K/V projections per key-block
    with attention for the hp=0 combos; pending AV matmuls burst at each
    key-block boundary (fills the PE while exp frees psum bufs, keeps HAM
    warm); each block's last keytile QK/exp is lagged into the next
    block's projection stretch so the exp engine has continuous runway.
    Phase B runs the hp=1 combos with a 2-keytile AV skew; the phase-A
    norms overlap phase B's start; the output projection accumulates in
    released attention psum banks with PSUM->SBUF casts on the ACT engine.
"""
import sys

sys.path.insert(0, "/opt/trn_rl_repo")

import numpy as np

import concourse.bass as bass
import concourse.tile as tile
from concourse import bacc, mybir
from concourse.bass_utils import run_bass_kernel_spmd

dt = mybir.dt
ts = bass.ts

B, N, L, D = 2, 1024, 4096, 1024
H, DH = 16, 64
HC = 4            # heads per core
CS = HC * DH      # 256 channel slice per core
SCALE = DH ** -0.5
N_CORES = 8
QB, KB = 2, 8     # query blocks of 512, key blocks of 512
DQC = 8           # contraction chunks of 128
KT = 32           # keytiles of 128
MASK_BIAS = -60.0

TRACE = False
LAST_EXEC_NS = None
_cache = {}


def _build():
    nc = bacc.Bacc("TRN2", target_bir_lowering=False, debug=False,
                   num_devices=N_CORES)
    bf = dt.bfloat16

    xTq = nc.dram_tensor("xTq", [D, N], bf, kind="ExternalInput").ap()
    xTkv = nc.dram_tensor("xTkv", [D, L], bf, kind="ExternalInput").ap()
    # weights pre-chunked on the host to [128, chunk, F] so the DMA reads
    # contiguous 4KB-per-partition lines instead of 512B strided rows
    wq = nc.dram_tensor("wq", [128, DQC, CS], bf, kind="ExternalInput").ap()
    wk = nc.dram_tensor("wk", [128, DQC, CS], bf, kind="ExternalInput").ap()
    wv = nc.dram_tensor("wv", [128, DQC, CS], bf, kind="ExternalInput").ap()
    wo = nc.dram_tensor("wo", [128, 2, D], bf, kind="ExternalInput").ap()
    bq2 = nc.dram_tensor("bq2", [128, 2], dt.float32, kind="ExternalInput").ap()
    bk2 = nc.dram_tensor("bk2", [128, 2], dt.float32, kind="ExternalInput").ap()
    bvb = nc.dram_tensor("bvb", [128, CS], dt.float32, kind="ExternalInput").ap()
    mb = nc.dram_tensor("mb", [128, KT], dt.float32, kind="ExternalInput").ap()
    out = nc.dram_tensor("out", [N, D], bf, kind="ExternalOutput").ap()

    with tile.TileContext(nc) as tc:
        _emit(nc, tc, xTq, xTkv, wq, wk, wv, wo, bq2, bk2, bvb, mb, out)
    nc.compile()
    return nc


def _emit(nc, tc, xTq, xTkv, wq, wk, wv, wo, bq2, bk2, bvb, mb, out):
    import contextlib

    bf = dt.bfloat16
    f32 = dt.float32
    ctx = contextlib.ExitStack()
    with ctx:
        persist = ctx.enter_context(tc.tile_pool(name="persist", bufs=1))
        xpool = ctx.enter_context(tc.tile_pool(name="xs", bufs=12))
        pT_pool = ctx.enter_context(tc.tile_pool(name="pT", bufs=18))
        rb_pool = ctx.enter_context(tc.tile_pool(name="rbs", bufs=2))
        outsb_pool = ctx.enter_context(tc.tile_pool(name="outsb", bufs=2))
        psT = ctx.enter_context(tc.tile_pool(name="psT", bufs=2, space="PSUM"))
        psOA_cm = tc.tile_pool(name="psOA", bufs=1, space="PSUM")
        psOA = psOA_cm.__enter__()
        lp = nc.allow_low_precision(reason="bf16 attention internals")
        lp.__enter__()

        def load_w3(name, src):
            # src: DRAM [128, d0, F] bf16 (host pre-chunked, contiguous).
            # Weights ride the ACT hardware DMA queue so they stream in
            # parallel with the x DMAs on the SP queue.
            r = persist.tile(list(src.shape), bf, tag=name, name=name)
            nc.scalar.dma_start(r[:], src)
            return r

        # ---- weights needed for the Q projection ------------------------
        wq_r = load_w3("wqr", wq)               # [128, 8, 256]
        bq_v = persist.tile([128, 2], f32, tag="bqv", name="bq_v")
        nc.scalar.dma_start(bq_v[:], bq2)
        mb_t = persist.tile([128, KT], f32, tag="mbt", name="mb_t")
        nc.scalar.dma_start(mb_t[:], mb)

        # ---- persistent activation tiles --------------------------------
        qT_sb = [persist.tile([128, N], bf, tag=f"qT{cc}", name=f"qT{cc}")
                 for cc in range(2)]
        kT_sb = [[persist.tile([128, 512], bf, tag=f"kT{cc}_{kb}",
                               name=f"kT{cc}_{kb}") for kb in range(KB)]
                 for cc in range(2)]
        va_sb = [persist.tile([128, HC, 65], bf, tag=f"va{kt}",
                              name=f"va{kt}") for kt in range(KT)]
        onT_sb = [persist.tile([128, N], bf, tag=f"onT{cc}",
                               name=f"onT{cc}") for cc in range(2)]

        # K weights on the SP queue so they land in parallel with wq (ACT)
        wk_r = persist.tile(list(wk.shape), bf, tag="wkr", name="wkr")
        nc.sync.dma_start(wk_r[:], wk)
        bk_v = persist.tile([128, 2], f32, tag="bkv", name="bk_v")
        nc.scalar.dma_start(bk_v[:], bk2)
        bv_b = persist.tile([128, CS], f32, tag="bvb", name="bv_b")
        nc.scalar.dma_start(bv_b[:], bvb)
        bv_b3 = bv_b[:].rearrange("p (h c) -> p h c", h=HC)
        ones128 = persist.tile([1, 128], bf, tag="o128", name="ones128")
        nc.vector.memset(ones128[:], 1.0)
        # augmented-V ones column (denominator row), set once
        for kt in range(KT):
            nc.vector.memset(va_sb[kt][:, :, 64:65], 1.0)

        # ---- attention helpers ------------------------------------------
        oPs = {}

        def open_oP(qb, hp, pool, sfx):
            oPs[(qb, hp)] = [
                pool.tile([128, 512], f32, tag=f"oP{qb}{hp}{h}{sfx}",
                          name=f"oP{qb}{hp}{h}{sfx}")
                for h in range(2)
            ]

        def attn_qk(qb, hp, kt):
            kb, kti = kt // 4, kt % 4
            sp = psT.tile([128, 1024], f32, tag="pp", name=f"sp{qb}{hp}{kt}")
            for h in range(2):
                nc.tensor.matmul(
                    sp[:, ts(h, 512)],
                    kT_sb[hp][kb][ts(h, 64), ts(kti, 128)],
                    qT_sb[hp][ts(h, 64), ts(qb, 512)],
                    start=True, stop=True,
                )
            pT = pT_pool.tile([128, 1024], bf, tag="pT", name=f"pT{qb}{hp}{kt}")
            nc.scalar.activation(pT[:], sp[:], mybir.ActivationFunctionType.Exp,
                                 scale=float(SCALE), bias=mb_t[:, kt:kt + 1])
            return pT

        def attn_av(qb, hp, kt, pT):
            oP = oPs[(qb, hp)]
            for h in range(2):
                nc.tensor.matmul(
                    oP[h][0:65, :], va_sb[kt][:, hp * 2 + h, :], pT[:, ts(h, 512)],
                    start=(kt == 0), stop=(kt == KT - 1),
                )

        def attn_norm(qb, hp):
            oP = oPs.pop((qb, hp))
            rb = psT.tile([128, 1024], f32, tag="pp", name=f"rb{qb}{hp}")
            rb_sb = rb_pool.tile([128, 1024], f32, tag="rbs", name=f"rbs{qb}{hp}")
            for h in range(2):
                den = rb_pool.tile([1, 512], f32, tag="den", name=f"den{qb}{hp}{h}")
                nc.vector.tensor_copy(den[:], oP[h][64:65, :])
                rdf = rb_pool.tile([1, 512], f32, tag="rdf", name=f"rdf{qb}{hp}{h}")
                # approx_fast needs an SBUF source (PSUM source returns garbage)
                nc.vector.reciprocal_approx_fast(rdf[:], den[:])
                rd = rb_pool.tile([1, 512], bf, tag="rd", name=f"rd{qb}{hp}{h}")
                nc.vector.tensor_copy(rd[:], rdf[:])
                nc.tensor.matmul(rb[:, ts(h, 512)], ones128[:], rd[:],
                                 start=True, stop=True)
            nc.vector.tensor_copy(rb_sb[:], rb[:])
            for h in range(2):
                nc.vector.tensor_mul(onT_sb[hp][ts(h, 64), ts(qb, 512)],
                                     oP[h][0:64, :], rb_sb[0:64, ts(h, 512)])

        # ---- fused Q projection + key-block-0 projections ---------------
        # Q(qb0) and K(kb0) stream together (both DMA queues + PE busy from
        # t=0), then Q(qb1) with V(kb0), so the first exp fires ~10us
        # earlier than a serial head.
        qp0 = psT.tile([128, 1024], f32, tag="pp", name="qp0")
        kp0 = psT.tile([128, 1024], f32, tag="pp", name="kp0")
        xks0 = []
        for dq in range(DQC):
            xfq = xpool.tile([128, 512], bf, tag="xs", name=f"xfq0_{dq}")
            eng = nc.sync if dq % 2 == 0 else nc.scalar
            eng.dma_start(xfq[:], xTq[ts(dq, 128), ts(0, 512)])
            for cc in range(2):
                nc.tensor.matmul(qp0[:, ts(cc, 512)], wq_r[:, dq, ts(cc, 128)],
                                 xfq[:], start=(dq == 0), stop=(dq == DQC - 1))
            xfk = xpool.tile([128, 512], bf, tag="xs", name=f"xfk0_{dq}")
            eng = nc.scalar if dq % 2 == 0 else nc.sync
            eng.dma_start(xfk[:], xTkv[ts(dq, 128), ts(0, 512)])
            xks0.append(xfk)
            for cc in range(2):
                nc.tensor.matmul(kp0[:, ts(cc, 512)], wk_r[:, dq, ts(cc, 128)],
                                 xfk[:], start=(dq == 0), stop=(dq == DQC - 1))
        for cc in range(2):
            nc.vector.tensor_scalar_add(qT_sb[cc][:, ts(0, 512)],
                                        qp0[:, ts(cc, 512)], bq_v[:, cc:cc + 1])
            nc.vector.tensor_scalar_add(kT_sb[cc][0][:], kp0[:, ts(cc, 512)],
                                        bk_v[:, cc:cc + 1])

        wv_r = load_w3("wvr", wv)
        # first attention keytile only needs qT(qb0) + kT(kb0): start the
        # exp chain now, while Q(qb1)/V(kb0) still stream
        head_qk = attn_qk(0, 0, 0)
        wo_r = load_w3("wor", wo)               # [128, 2, 1024]

        qp1 = psT.tile([128, 1024], f32, tag="pp", name="qp1")
        vp0 = psT.tile([128, 1024], f32, tag="pp", name="vp0")
        for dq in range(DQC):
            xfq = xpool.tile([128, 512], bf, tag="xs", name=f"xfq1_{dq}")
            eng = nc.sync if dq % 2 == 0 else nc.scalar
            eng.dma_start(xfq[:], xTq[ts(dq, 128), ts(1, 512)])
            for cc in range(2):
                nc.tensor.matmul(qp1[:, ts(cc, 512)], wq_r[:, dq, ts(cc, 128)],
                                 xfq[:], start=(dq == 0), stop=(dq == DQC - 1))
            for t in range(4):
                nc.tensor.matmul(vp0[:, ts(t, 256)], xks0[dq][:, ts(t, 128)],
                                 wv_r[:, dq, :],
                                 start=(dq == 0 and t % 2 == 0),
                                 stop=(dq == DQC - 1 and t % 2 == 1))
        for cc in range(2):
            nc.vector.tensor_scalar_add(qT_sb[cc][:, ts(1, 512)],
                                        qp1[:, ts(cc, 512)], bq_v[:, cc:cc + 1])
        for t in range(4):
            src = vp0[:, ts(t, 256)].rearrange("p (h c) -> p h c", h=HC)
            nc.vector.tensor_add(va_sb[t][:, :, 0:64], src, bv_b3)

        # ---- phase A: K/V projections + attention on hp=0 (both qb) -----
        open_oP(0, 0, psOA, "a")
        open_oP(1, 0, psOA, "a")
        pend_av = []

        def drip():
            if pend_av:
                attn_av(*pend_av.pop(0))

        lag_qk = []
        pend_av.append((0, 0, 0, head_qk))
        for t in range(4):
            if t == 3:
                lag_qk = [(qb, 0, t) for qb in range(QB)]
                break
            for qb in range(QB):
                if (qb, t) == (0, 0):
                    continue
                pend_av.append((qb, 0, t, attn_qk(qb, 0, t)))
        for kb in range(1, KB):
            # dense AV burst first: these are runnable (their exps are done
            # or nearly done) and fill the PE while the exp chain frees the
            # projection psum bufs — no boundary stall, HAM stays warm
            while pend_av:
                drip()
            kp = psT.tile([128, 1024], f32, tag="pp", name=f"kp{kb}")
            xks = []
            for dq in range(DQC):
                xf = xpool.tile([128, 512], bf, tag="xs", name=f"xfk{kb}_{dq}")
                eng = nc.sync if dq % 2 == 0 else nc.scalar
                eng.dma_start(xf[:], xTkv[ts(dq, 128), ts(kb, 512)])
                xks.append(xf)
                for cc in range(2):
                    nc.tensor.matmul(kp[:, ts(cc, 512)], wk_r[:, dq, ts(cc, 128)],
                                     xf[:], start=(dq == 0), stop=(dq == DQC - 1))
            for cc in range(2):
                nc.vector.tensor_scalar_add(kT_sb[cc][kb][:], kp[:, ts(cc, 512)],
                                            bk_v[:, cc:cc + 1])

            # one lagged QK before vp (vp then inherits kp's psum slot
            # instead of waiting on an exp), the second after vp's matmuls
            if lag_qk:
                lqb, lhp, lkt = lag_qk.pop(0)
                pend_av.append((lqb, lhp, lkt, attn_qk(lqb, lhp, lkt)))

            vp = psT.tile([128, 1024], f32, tag="pp", name=f"vp{kb}")
            for dq in range(DQC):
                for t in range(4):
                    # start clears has_written for the whole 2KB psum bank, so
                    # only the first matmul touching each bank may set it
                    nc.tensor.matmul(vp[:, ts(t, 256)], xks[dq][:, ts(t, 128)],
                                     wv_r[:, dq, :],
                                     start=(dq == 0 and t % 2 == 0),
                                     stop=(dq == DQC - 1 and t % 2 == 1))
            for (lqb, lhp, lkt) in lag_qk:
                pend_av.append((lqb, lhp, lkt, attn_qk(lqb, lhp, lkt)))
            lag_qk = []
            for t in range(4):
                kt = kb * 4 + t
                src = vp[:, ts(t, 256)].rearrange("p (h c) -> p h c", h=HC)
                nc.vector.tensor_add(va_sb[kt][:, :, 0:64], src, bv_b3)

            for t in range(4):
                kt = kb * 4 + t
                if t == 3:
                    lag_qk = [(qb, 0, kt) for qb in range(QB)]
                    break
                for qb in range(QB):
                    pT = attn_qk(qb, 0, kt)
                    pend_av.append((qb, 0, kt, pT))

        # ---- phase B: attention on hp=1 (both qb ragged) ----------------
        # phase B's QK/exp chain starts right away; the last key-block's
        # hp=0 AV matmuls drain under it, then the phase-A norms run and
        # release the psOA banks for phase B's accumulators.
        for (lqb, lhp, lkt) in lag_qk:
            pend_av.append((lqb, lhp, lkt, attn_qk(lqb, lhp, lkt)))
        lag_qk = []
        for kt in range(3):
            for qb in range(QB):
                pend_av.append((qb, 1, kt, attn_qk(qb, 1, kt)))
            for _ in range(4):
                if pend_av and pend_av[0][1] == 0 and len(pend_av) > 4:
                    drip()
        while pend_av and pend_av[0][1] == 0:
            drip()
        attn_norm(0, 0)
        attn_norm(1, 0)
        psOA_cm.__exit__(None, None, None)
        psOB_cm = tc.tile_pool(name="psOB", bufs=1, space="PSUM")
        psOB = psOB_cm.__enter__()
        open_oP(0, 1, psOB, "b")
        open_oP(1, 1, psOB, "b")
        for kt in range(3, KT):
            for qb in range(QB):
                pend_av.append((qb, 1, kt, attn_qk(qb, 1, kt)))
            while len(pend_av) > 4:
                drip()
        while pend_av:
            drip()

        def oproj(qt):
            # accumulate in the released (0,1) oP banks
            ops = []
            for eb in range(2):
                op = psOB.tile([128, 512], f32, tag=f"oP01{eb}b",
                               name=f"op{qt}_{eb}")
                ops.append(op)
                for cc in range(2):
                    nc.tensor.matmul(op[:, :], onT_sb[cc][:, ts(qt, 128)],
                                     wo_r[:, cc, ts(eb, 512)],
                                     start=(cc == 0), stop=(cc == 1))
            osb = outsb_pool.tile([128, 1024], bf, tag="osb", name=f"osb{qt}")
            for eb in range(2):
                # ACT engine is idle in the tail; DVE runs the norm chains
                nc.scalar.copy(osb[:, ts(eb, 512)], ops[eb][:])
            nc.scalar.dma_start(out[ts(qt, 128), :], osb[:])

        # tail: norms feed the output projection; oproj PE work overlaps
        # the norm DVE chains
        attn_norm(0, 1)
        attn_norm(1, 1)
        for qt in range(8):
            oproj(qt)

        psOB_cm.__exit__(None, None, None)
        lp.__exit__(None, None, None)


def kernel(x_q, x_kv, pad_mask, Wq, bq, Wk, bk, Wv, bv, Wo, bo):
    global LAST_EXEC_NS
    import ml_dtypes
    bf16 = ml_dtypes.bfloat16

    x_q = np.asarray(x_q, np.float32)
    x_kv = np.asarray(x_kv, np.float32)
    pad_mask = np.asarray(pad_mask)
    Wq, bq = np.asarray(Wq, np.float32), np.asarray(bq, np.float32)
    Wk, bk = np.asarray(Wk, np.float32), np.asarray(bk, np.float32)
    Wv, bv = np.asarray(Wv, np.float32), np.asarray(bv, np.float32)
    Wo, bo = np.asarray(Wo, np.float32), np.asarray(bo, np.float32)

    if "nc" not in _cache:
        _cache["nc"] = _build()
    nc = _cache["nc"]

    xTq_b = [np.ascontiguousarray(x_q[b].T.astype(bf16)) for b in range(B)]
    xTkv_b = [np.ascontiguousarray(x_kv[b].T.astype(bf16)) for b in range(B)]
    mb_b = []
    for b in range(B):
        m = np.where(pad_mask[b], np.float32(MASK_BIAS), np.float32(0.0))
        mb_b.append(np.ascontiguousarray(m.reshape(KT, 128).T.astype(np.float32)))

    in_maps = []
    for c in range(N_CORES):
        b, g = c // 4, c % 4
        hs = g * CS
        in_maps.append({
            "xTq": xTq_b[b],
            "xTkv": xTkv_b[b],
            "wq": np.ascontiguousarray(
                Wq[:, hs:hs + CS].reshape(DQC, 128, CS).transpose(1, 0, 2)
                .astype(bf16)),
            "wk": np.ascontiguousarray(
                Wk[:, hs:hs + CS].reshape(DQC, 128, CS).transpose(1, 0, 2)
                .astype(bf16)),
            "wv": np.ascontiguousarray(
                Wv[:, hs:hs + CS].reshape(DQC, 128, CS).transpose(1, 0, 2)
                .astype(bf16)),
            "wo": np.ascontiguousarray(
                Wo[hs:hs + CS, :].reshape(2, 128, D).transpose(1, 0, 2)
                .astype(bf16)),
            "bq2": np.ascontiguousarray(bq[hs:hs + CS].reshape(2, 128).T),
            "bk2": np.ascontiguousarray(bk[hs:hs + CS].reshape(2, 128).T),
            "bvb": np.ascontiguousarray(
                np.broadcast_to(bv[hs:hs + CS], (128, CS)).astype(np.float32)),
            "mb": mb_b[b],
        })

    res = run_bass_kernel_spmd(nc, in_maps, list(range(N_CORES)), trace=TRACE)
    LAST_EXEC_NS = res.exec_time_ns

    outp = np.zeros((B, N, D), np.float32)
    for c in range(N_CORES):
        outp[c // 4] += res.results[c]["out"].astype(np.float32)
    outp += bo
    return outp


# revision 27
# speedup vs baseline: 1.1604x; 1.1604x over previous
"""Multi-head cross-attention (B=2, N=1024, L=4096, D=1024, H=16) on 8 trn2
NeuronCores — bf16 pipeline (~257-261us, vs 350-395us fp32r baseline).

Sharding: batch x head-group data/tensor parallel. Core c handles batch
c//4 and heads 4*(c%4) .. 4*(c%4)+3 (weight columns sliced per head group,
Wo row-sliced; partial outputs summed on the host during unsharding).

Design notes:
  - all matmul operands bf16 (fp32 PSUM accumulate): fp32 moving operands
    stream at 2 cycles/col on the PE xbus, bf16 at 1 — halves matmul time
    and DMA traffic, and the DMA'd bf16 tiles feed matmuls directly (no
    fp32->fp32r DVE casts). rel_absmax ~5.6e-3 vs the fp32 reference.
  - padding mask applied as a per-key additive bias (-60) inside the exp
    activation (bias is a [128,1] per-partition AP), so V needs no keep
    premultiply; the augmented-V ones column provides the softmax
    denominator (row 64 of the attention-output PSUM accumulator).
  - q/k biases folded into the PSUM->SBUF copies (tensor_scalar_add with a
    per-partition bias vector); v bias added during the va build (DVE
    tensor_tensor add); no bias matmuls.
  - weights pre-chunked on the host to partition-major [128, chunk, F] so
    weight DMA reads contiguous 4KB lines; weights and half the x tiles
    ride the ACT hardware DMA queue, the rest the SP queue (2 queues in
    parallel, ~200GB/s each).
  - software pipeline: phase A interleaves K/V projections per key-block
    with attention for the hp=0 combos; pending AV matmuls burst at each
    key-block boundary (fills the PE while exp frees psum bufs, keeps HAM
    warm); each block's last keytile QK/exp is lagged into the next
    block's projection stretch so the exp engine has continuous runway.
    Phase B runs the hp=1 combos with a 2-keytile AV skew; the phase-A
    norms overlap phase B's start; the output projection accumulates in
    released attention psum banks with PSUM->SBUF casts on the ACT engine.
"""
import sys

sys.path.insert(0, "/opt/trn_rl_repo")

import numpy as np

import concourse.bass as bass
import concourse.tile as tile
from concourse import bacc, mybir
from concourse.bass_utils import run_bass_kernel_spmd

dt = mybir.dt
ts = bass.ts

B, N, L, D = 2, 1024, 4096, 1024
H, DH = 16, 64
HC = 4            # heads per core
CS = HC * DH      # 256 channel slice per core
SCALE = DH ** -0.5
N_CORES = 8
QB, KB = 2, 8     # query blocks of 512, key blocks of 512
DQC = 8           # contraction chunks of 128
KT = 32           # keytiles of 128
MASK_BIAS = -60.0

TRACE = False
LAST_EXEC_NS = None
_cache = {}


def _build():
    nc = bacc.Bacc("TRN2", target_bir_lowering=False, debug=False,
                   num_devices=N_CORES)
    bf = dt.bfloat16

    xTq = nc.dram_tensor("xTq", [D, N], bf, kind="ExternalInput").ap()
    xTkv = nc.dram_tensor("xTkv", [D, L], bf, kind="ExternalInput").ap()
    # weights pre-chunked on the host to [128, chunk, F] so the DMA reads
    # contiguous 4KB-per-partition lines instead of 512B strided rows
    wq = nc.dram_tensor("wq", [128, DQC, CS], bf, kind="ExternalInput").ap()
    wk = nc.dram_tensor("wk", [128, DQC, CS], bf, kind="ExternalInput").ap()
    wv = nc.dram_tensor("wv", [128, DQC, CS], bf, kind="ExternalInput").ap()
    wo = nc.dram_tensor("wo", [128, 2, D], bf, kind="ExternalInput").ap()
    bq2 = nc.dram_tensor("bq2", [128, 2], dt.float32, kind="ExternalInput").ap()
    bk2 = nc.dram_tensor("bk2", [128, 2], dt.float32, kind="ExternalInput").ap()
    bvb = nc.dram_tensor("bvb", [128, CS], dt.float32, kind="ExternalInput").ap()
    mb = nc.dram_tensor("mb", [128, KT], dt.float32, kind="ExternalInput").ap()
    out = nc.dram_tensor("out", [N, D], bf, kind="ExternalOutput").ap()

    with tile.TileContext(nc) as tc:
        _emit(nc, tc, xTq, xTkv, wq, wk, wv, wo, bq2, bk2, bvb, mb, out)
    nc.compile()
    return nc


def _emit(nc, tc, xTq, xTkv, wq, wk, wv, wo, bq2, bk2, bvb, mb, out):
    import contextlib

    bf = dt.bfloat16
    f32 = dt.float32
    ctx = contextlib.ExitStack()
    with ctx:
        persist = ctx.enter_context(tc.tile_pool(name="persist", bufs=1))
        xpool = ctx.enter_context(tc.tile_pool(name="xs", bufs=12))
        pT_pool = ctx.enter_context(tc.tile_pool(name="pT", bufs=18))
        rb_pool = ctx.enter_context(tc.tile_pool(name="rbs", bufs=2))
        outsb_pool = ctx.enter_context(tc.tile_pool(name="outsb", bufs=2))
        psT = ctx.enter_context(tc.tile_pool(name="psT", bufs=2, space="PSUM"))
        psPR = ctx.enter_context(tc.tile_pool(name="psPR", bufs=1, space="PSUM"))
        psOA_cm = tc.tile_pool(name="psOA", bufs=1, space="PSUM")
        psOA = psOA_cm.__enter__()
        lp = nc.allow_low_precision(reason="bf16 attention internals")
        lp.__enter__()

        def load_w3(name, src):
            # src: DRAM [128, d0, F] bf16 (host pre-chunked, contiguous).
            # Weights ride the ACT hardware DMA queue so they stream in
            # parallel with the x DMAs on the SP queue.
            r = persist.tile(list(src.shape), bf, tag=name, name=name)
            nc.scalar.dma_start(r[:], src)
            return r

        # ---- weights needed for the Q projection ------------------------
        wq_r = load_w3("wqr", wq)               # [128, 8, 256]
        bq_v = persist.tile([128, 2], f32, tag="bqv", name="bq_v")
        nc.scalar.dma_start(bq_v[:], bq2)
        mb_t = persist.tile([128, KT], f32, tag="mbt", name="mb_t")
        nc.scalar.dma_start(mb_t[:], mb)

        # ---- persistent activation tiles --------------------------------
        qT_sb = [persist.tile([128, N], bf, tag=f"qT{cc}", name=f"qT{cc}")
                 for cc in range(2)]
        kT_sb = [[persist.tile([128, 512], bf, tag=f"kT{cc}_{kb}",
                               name=f"kT{cc}_{kb}") for kb in range(KB)]
                 for cc in range(2)]
        va_sb = [persist.tile([128, HC, DH], bf, tag=f"va{kt}",
                              name=f"va{kt}") for kt in range(KT)]
        onT_sb = [persist.tile([128, N], bf, tag=f"onT{cc}",
                               name=f"onT{cc}") for cc in range(2)]

        # ---- Q projection ----------------------------------------------
        for qb in range(QB):
            qp = psT.tile([128, 1024], f32, tag="pp", name=f"qp{qb}")
            for dq in range(DQC):
                xf = xpool.tile([128, 512], bf, tag="xs", name=f"xfq{qb}_{dq}")
                eng = nc.sync if dq % 2 == 0 else nc.scalar
                eng.dma_start(xf[:], xTq[ts(dq, 128), ts(qb, 512)])
                for cc in range(2):
                    nc.tensor.matmul(qp[:, ts(cc, 512)], wq_r[:, dq, ts(cc, 128)],
                                     xf[:], start=(dq == 0), stop=(dq == DQC - 1))
            for cc in range(2):
                nc.vector.tensor_scalar_add(qT_sb[cc][:, ts(qb, 512)],
                                            qp[:, ts(cc, 512)], bq_v[:, cc:cc + 1])

        # remaining weights (DMA priority after the q-projection inputs)
        wk_r = load_w3("wkr", wk)
        wv_r = load_w3("wvr", wv)
        wo_r = load_w3("wor", wo)               # [128, 2, 1024]
        bk_v = persist.tile([128, 2], f32, tag="bkv", name="bk_v")
        nc.scalar.dma_start(bk_v[:], bk2)
        bv_b = persist.tile([128, CS], f32, tag="bvb", name="bv_b")
        nc.scalar.dma_start(bv_b[:], bvb)
        bv_b3 = bv_b[:].rearrange("p (h c) -> p h c", h=HC)
        ones128 = persist.tile([1, 128], bf, tag="o128", name="ones128")
        nc.vector.memset(ones128[:], 1.0)
        ones_col = persist.tile([128, 1], bf, tag="ocol", name="ones_col")
        nc.vector.memset(ones_col[:], 1.0)

        # ---- attention helpers ------------------------------------------
        oPs = {}
        dens = {}

        def open_oP(qb, hp, pool, sfx):
            # both heads share one bank: h0 at partitions 0:64, h1 at 64:128
            oPs[(qb, hp)] = pool.tile([128, 512], f32, tag=f"oP{qb}{hp}{sfx}",
                                      name=f"oP{qb}{hp}{sfx}")

        def open_den(hp, pool, sfx):
            # four denominator streams (qb, h) at partitions 0/32/64/96
            dens[hp] = pool.tile([128, 512], f32, tag=f"dn{sfx}",
                                 name=f"dn{sfx}")

        def attn_qk(qb, hp, kt):
            kb, kti = kt // 4, kt % 4
            sp = psT.tile([128, 1024], f32, tag="pp", name=f"sp{qb}{hp}{kt}")
            for h in range(2):
                nc.tensor.matmul(
                    sp[:, ts(h, 512)],
                    kT_sb[hp][kb][ts(h, 64), ts(kti, 128)],
                    qT_sb[hp][ts(h, 64), ts(qb, 512)],
                    start=True, stop=True,
                )
            pT = pT_pool.tile([128, 1024], bf, tag="pT", name=f"pT{qb}{hp}{kt}")
            nc.scalar.activation(pT[:], sp[:], mybir.ActivationFunctionType.Exp,
                                 scale=float(SCALE), bias=mb_t[:, kt:kt + 1])
            return pT

        def attn_av(qb, hp, kt, pT):
            # h0/h1 into col groups (0,0)/(0,64) of one bank — concurrent.
            # only the first matmul touching the bank may set start (it
            # clears has_written bank-wide); later first-writes then store.
            oP = oPs[(qb, hp)]
            for h in range(2):
                nc.tensor.matmul(
                    oP[ts(h, 64), :], va_sb[kt][:, hp * 2 + h, :],
                    pT[:, ts(h, 512)],
                    start=(kt == 0 and h == 0), stop=(kt == KT - 1),
                    skip_group_check=True,
                )
            dn = dens[hp]
            for h in range(2):
                strm = 32 * (qb * 2 + h)
                nc.tensor.matmul(
                    dn[strm:strm + 1, :], ones_col[:], pT[:, ts(h, 512)],
                    start=(kt == 0 and qb == 0 and h == 0),
                    stop=(kt == KT - 1), skip_group_check=True,
                    tile_position=(0, strm),
                )

        def attn_norm(qb, hp):
            oP = oPs.pop((qb, hp))
            dn = dens[hp]
            for h in range(2):
                strm = 32 * (qb * 2 + h)
                den = rb_pool.tile([1, 512], f32, tag="den", name=f"den{qb}{hp}{h}")
                nc.vector.tensor_copy(den[:], dn[strm:strm + 1, :])
                rdf = rb_pool.tile([1, 512], f32, tag="rdf", name=f"rdf{qb}{hp}{h}")
                # approx_fast needs an SBUF source (PSUM source returns garbage)
                nc.vector.reciprocal_approx_fast(rdf[:], den[:])
                rd = rb_pool.tile([1, 512], bf, tag="rd", name=f"rd{qb}{hp}{h}")
                nc.vector.tensor_copy(rd[:], rdf[:])
                rb = psPR.tile([128, 512], f32, tag="prj", name=f"rb{qb}{hp}{h}")
                nc.tensor.matmul(rb[:, :], ones128[:], rd[:],
                                 start=True, stop=True)
                rb_sb = rb_pool.tile([128, 512], f32, tag="rbs",
                                     name=f"rbs{qb}{hp}{h}")
                nc.vector.tensor_copy(rb_sb[:], rb[:])
                nc.vector.tensor_mul(onT_sb[hp][ts(h, 64), ts(qb, 512)],
                                     oP[ts(h, 64), :], rb_sb[0:64, :])

        # ---- phase A: K/V projections + attention on hp=0 (both qb) -----
        open_oP(0, 0, psOA, "a")
        open_oP(1, 0, psOA, "a")
        open_den(0, psOA, "A")
        pend_av = []

        def drip():
            if pend_av:
                attn_av(*pend_av.pop(0))

        lag_qk = []

        def lag_drip(n):
            # emit up to n lagged QK/exp units; the score tiles have a
            # dedicated psum pool now, so these interleave freely with the
            # projection matmuls and keep the exp engine continuously fed
            for _ in range(n):
                if lag_qk:
                    lqb, lhp, lkt = lag_qk.pop(0)
                    pend_av.append((lqb, lhp, lkt, attn_qk(lqb, lhp, lkt)))

        for kb in range(KB):
            # dense AV+den burst first: these are runnable and fill the PE
            # at the block boundary, keeping HAM warm
            while pend_av:
                drip()
            xks = []
            for dq in range(DQC):
                xf = xpool.tile([128, 512], bf, tag="xs", name=f"xfk{kb}_{dq}")
                eng = nc.sync if dq % 2 == 0 else nc.scalar
                eng.dma_start(xf[:], xTkv[ts(dq, 128), ts(kb, 512)])
                xks.append(xf)
            for half in range(2):
                kp = psPR.tile([128, 512], f32, tag="prj",
                               name=f"kp{kb}_{half}")
                for dq in range(DQC):
                    for cc in range(2):
                        nc.tensor.matmul(kp[:, ts(cc, 256)],
                                         wk_r[:, dq, ts(cc, 128)],
                                         xks[dq][:, ts(half, 256)],
                                         start=(dq == 0 and cc == 0),
                                         stop=(dq == DQC - 1 and cc == 1))
                lag_drip(2)
                for cc in range(2):
                    nc.vector.tensor_scalar_add(
                        kT_sb[cc][kb][:, ts(half, 256)], kp[:, ts(cc, 256)],
                        bk_v[:, cc:cc + 1])
            for vh in range(2):
                vp = psPR.tile([128, 512], f32, tag="prj",
                               name=f"vp{kb}_{vh}")
                for dq in range(DQC):
                    for t2 in range(2):
                        nc.tensor.matmul(vp[:, ts(t2, 256)],
                                         xks[dq][:, ts(vh * 2 + t2, 128)],
                                         wv_r[:, dq, :],
                                         start=(dq == 0 and t2 == 0),
                                         stop=(dq == DQC - 1 and t2 == 1))
                lag_drip(2)
                for t2 in range(2):
                    kt = kb * 4 + vh * 2 + t2
                    src = vp[:, ts(t2, 256)].rearrange("p (h c) -> p h c", h=HC)
                    nc.vector.tensor_add(va_sb[kt][:, :, :], src, bv_b3)
            # this block's attention lags into the next block's projections
            lag_qk += [(qb, 0, kb * 4 + t) for t in range(4)
                       for qb in range(QB)]

        # ---- phase B: attention on hp=1 (both qb ragged) ----------------
        # phase B's QK/exp chain starts right away; the last key-block's
        # hp=0 AV matmuls drain under it, then the phase-A norms run and
        # release the psOA banks for phase B's accumulators.
        for (lqb, lhp, lkt) in lag_qk:
            pend_av.append((lqb, lhp, lkt, attn_qk(lqb, lhp, lkt)))
        lag_qk = []
        for kt in range(3):
            for qb in range(QB):
                pend_av.append((qb, 1, kt, attn_qk(qb, 1, kt)))
            for _ in range(4):
                if pend_av and pend_av[0][1] == 0 and len(pend_av) > 4:
                    drip()
        while pend_av and pend_av[0][1] == 0:
            drip()
        attn_norm(0, 0)
        attn_norm(1, 0)
        psOA_cm.__exit__(None, None, None)
        psOB_cm = tc.tile_pool(name="psOB", bufs=1, space="PSUM")
        psOB = psOB_cm.__enter__()
        open_oP(0, 1, psOB, "b")
        open_oP(1, 1, psOB, "b")
        open_den(1, psOB, "B")
        for kt in range(3, KT):
            for qb in range(QB):
                pend_av.append((qb, 1, kt, attn_qk(qb, 1, kt)))
            while len(pend_av) > 4:
                drip()
        while pend_av:
            drip()

        def oproj(qt):
            # accumulate in the released (0,1) oP banks
            ops = [psOB.tile([128, 512], f32, tag="oP01b", name=f"op{qt}_0"),
                   psPR.tile([128, 512], f32, tag="prj", name=f"op{qt}_1")]
            for eb in range(2):
                for cc in range(2):
                    nc.tensor.matmul(ops[eb][:, :], onT_sb[cc][:, ts(qt, 128)],
                                     wo_r[:, cc, ts(eb, 512)],
                                     start=(cc == 0), stop=(cc == 1))
            osb = outsb_pool.tile([128, 1024], bf, tag="osb", name=f"osb{qt}")
            for eb in range(2):
                # ACT engine is idle in the tail; DVE runs the norm chains
                nc.scalar.copy(osb[:, ts(eb, 512)], ops[eb][:])
            nc.scalar.dma_start(out[ts(qt, 128), :], osb[:])

        # tail: norms feed the output projection; oproj PE work overlaps
        # the norm DVE chains
        attn_norm(0, 1)
        for qt in range(4):
            oproj(qt)
        attn_norm(1, 1)
        for qt in range(4, 8):
            oproj(qt)

        psOB_cm.__exit__(None, None, None)
        lp.__exit__(None, None, None)


def kernel(x_q, x_kv, pad_mask, Wq, bq, Wk, bk, Wv, bv, Wo, bo):
    global LAST_EXEC_NS
    import ml_dtypes
    bf16 = ml_dtypes.bfloat16

    x_q = np.asarray(x_q, np.float32)
    x_kv = np.asarray(x_kv, np.float32)
    pad_mask = np.asarray(pad_mask)
    Wq, bq = np.asarray(Wq, np.float32), np.asarray(bq, np.float32)
    Wk, bk = np.asarray(Wk, np.float32), np.asarray(bk, np.float32)
    Wv, bv = np.asarray(Wv, np.float32), np.asarray(bv, np.float32)
    Wo, bo = np.asarray(Wo, np.float32), np.asarray(bo, np.float32)

    if "nc" not in _cache:
        _cache["nc"] = _build()
    nc = _cache["nc"]

    xTq_b = [np.ascontiguousarray(x_q[b].T.astype(bf16)) for b in range(B)]
    xTkv_b = [np.ascontiguousarray(x_kv[b].T.astype(bf16)) for b in range(B)]
    mb_b = []
    for b in range(B):
        m = np.where(pad_mask[b], np.float32(MASK_BIAS), np.float32(0.0))
        mb_b.append(np.ascontiguousarray(m.reshape(KT, 128).T.astype(np.float32)))

    in_maps = []
    for c in range(N_CORES):
        b, g = c // 4, c % 4
        hs = g * CS
        in_maps.append({
            "xTq": xTq_b[b],
            "xTkv": xTkv_b[b],
            "wq": np.ascontiguousarray(
                Wq[:, hs:hs + CS].reshape(DQC, 128, CS).transpose(1, 0, 2)
                .astype(bf16)),
            "wk": np.ascontiguousarray(
                Wk[:, hs:hs + CS].reshape(DQC, 128, CS).transpose(1, 0, 2)
                .astype(bf16)),
            "wv": np.ascontiguousarray(
                Wv[:, hs:hs + CS].reshape(DQC, 128, CS).transpose(1, 0, 2)
                .astype(bf16)),
            "wo": np.ascontiguousarray(
                Wo[hs:hs + CS, :].reshape(2, 128, D).transpose(1, 0, 2)
                .astype(bf16)),
            "bq2": np.ascontiguousarray(bq[hs:hs + CS].reshape(2, 128).T),
            "bk2": np.ascontiguousarray(bk[hs:hs + CS].reshape(2, 128).T),
            "bvb": np.ascontiguousarray(
                np.broadcast_to(bv[hs:hs + CS], (128, CS)).astype(np.float32)),
            "mb": mb_b[b],
        })

    res = run_bass_kernel_spmd(nc, in_maps, list(range(N_CORES)), trace=TRACE)
    LAST_EXEC_NS = res.exec_time_ns

    outp = np.zeros((B, N, D), np.float32)
    for c in range(N_CORES):
        outp[c // 4] += res.results[c]["out"].astype(np.float32)
    outp += bo
    return outp


# revision 29
# speedup vs baseline: 1.1734x; 1.0112x over previous
"""Multi-head cross-attention (B=2, N=1024, L=4096, D=1024, H=16) on 8 trn2
NeuronCores — bf16 v2.

Sharding: batch x head-group data/tensor parallel. Core c handles batch
c//4 and heads 4*(c%4) .. 4*(c%4)+3 (weight columns sliced per head group,
Wo row-sliced; partial outputs summed on the host during unsharding).

v2 changes vs the fp32r baseline:
  - all matmul operands bf16 (fp32 PSUM accumulate): fp32 moving operands
    stream at 2 cycles/col on the PE xbus, bf16 at 1 — halves matmul time
    and DMA traffic, and the DMA'd bf16 tiles feed matmuls directly (no
    fp32->fp32r DVE casts).
  - padding mask applied as a per-key additive bias (-60) inside the exp
    activation (bias is a [128,1] per-partition AP), so V needs no keep
    premultiply; the augmented-V ones column provides the denominator.
  - q/k biases folded into the PSUM->SBUF copies (tensor_scalar_add with a
    per-partition bias vector); v bias added during the va build.
  - AV matmuls for key-block kb are dripped into kb+1's projection matmuls
    so the PE never stalls on the exp (ACT) latency.
"""
import sys

sys.path.insert(0, "/opt/trn_rl_repo")

import numpy as np

import concourse.bass as bass
import concourse.tile as tile
from concourse import bacc, mybir
from concourse.bass_utils import run_bass_kernel_spmd

dt = mybir.dt
ts = bass.ts

B, N, L, D = 2, 1024, 4096, 1024
H, DH = 16, 64
HC = 4            # heads per core
CS = HC * DH      # 256 channel slice per core
SCALE = DH ** -0.5
N_CORES = 8
QB, KB = 2, 8     # query blocks of 512, key blocks of 512
DQC = 8           # contraction chunks of 128
KT = 32           # keytiles of 128
MASK_BIAS = -60.0

TRACE = False
LAST_EXEC_NS = None
_cache = {}


def _build():
    nc = bacc.Bacc("TRN2", target_bir_lowering=False, debug=False,
                   num_devices=N_CORES)
    bf = dt.bfloat16

    xTq = nc.dram_tensor("xTq", [D, N], bf, kind="ExternalInput").ap()
    xTkv = nc.dram_tensor("xTkv", [D, L], bf, kind="ExternalInput").ap()
    # weights pre-chunked on the host to [128, chunk, F] so the DMA reads
    # contiguous 4KB-per-partition lines instead of 512B strided rows
    wq = nc.dram_tensor("wq", [128, DQC, CS], bf, kind="ExternalInput").ap()
    wk = nc.dram_tensor("wk", [128, DQC, CS], bf, kind="ExternalInput").ap()
    wv = nc.dram_tensor("wv", [128, DQC, CS], bf, kind="ExternalInput").ap()
    wo = nc.dram_tensor("wo", [128, 2, D], bf, kind="ExternalInput").ap()
    bq2 = nc.dram_tensor("bq2", [128, 2], dt.float32, kind="ExternalInput").ap()
    bk2 = nc.dram_tensor("bk2", [128, 2], dt.float32, kind="ExternalInput").ap()
    bvb = nc.dram_tensor("bvb", [128, CS], dt.float32, kind="ExternalInput").ap()
    mb = nc.dram_tensor("mb", [128, KT], dt.float32, kind="ExternalInput").ap()
    out = nc.dram_tensor("out", [N, D], bf, kind="ExternalOutput").ap()

    with tile.TileContext(nc) as tc:
        _emit(nc, tc, xTq, xTkv, wq, wk, wv, wo, bq2, bk2, bvb, mb, out)
    nc.compile()
    return nc


def _emit(nc, tc, xTq, xTkv, wq, wk, wv, wo, bq2, bk2, bvb, mb, out):
    import contextlib

    bf = dt.bfloat16
    f32 = dt.float32
    ctx = contextlib.ExitStack()
    with ctx:
        persist = ctx.enter_context(tc.tile_pool(name="persist", bufs=1))
        xpool = ctx.enter_context(tc.tile_pool(name="xs", bufs=12))
        pT_pool = ctx.enter_context(tc.tile_pool(name="pT", bufs=18))
        rb_pool = ctx.enter_context(tc.tile_pool(name="rbs", bufs=2))
        outsb_pool = ctx.enter_context(tc.tile_pool(name="outsb", bufs=2))
        psT = ctx.enter_context(tc.tile_pool(name="psT", bufs=2, space="PSUM"))
        psOA_cm = tc.tile_pool(name="psOA", bufs=1, space="PSUM")
        psOA = psOA_cm.__enter__()
        lp = nc.allow_low_precision(reason="bf16 attention internals")
        lp.__enter__()

        def load_w3(name, src):
            # src: DRAM [128, d0, F] bf16 (host pre-chunked, contiguous).
            # Weights ride the ACT hardware DMA queue so they stream in
            # parallel with the x DMAs on the SP queue.
            r = persist.tile(list(src.shape), bf, tag=name, name=name)
            nc.scalar.dma_start(r[:], src)
            return r

        # ---- weights needed for the Q projection ------------------------
        wq_r = load_w3("wqr", wq)               # [128, 8, 256]
        bq_v = persist.tile([128, 2], f32, tag="bqv", name="bq_v")
        nc.scalar.dma_start(bq_v[:], bq2)
        mb_t = persist.tile([128, KT], f32, tag="mbt", name="mb_t")
        nc.scalar.dma_start(mb_t[:], mb)

        # ---- persistent activation tiles --------------------------------
        qT_sb = [persist.tile([128, N], bf, tag=f"qT{cc}", name=f"qT{cc}")
                 for cc in range(2)]
        kT_sb = [[persist.tile([128, 512], bf, tag=f"kT{cc}_{kb}",
                               name=f"kT{cc}_{kb}") for kb in range(KB)]
                 for cc in range(2)]
        va_sb = [persist.tile([128, HC, 65], bf, tag=f"va{kt}",
                              name=f"va{kt}") for kt in range(KT)]
        onT_sb = [persist.tile([128, N], bf, tag=f"onT{cc}",
                               name=f"onT{cc}") for cc in range(2)]

        # ---- Q projection ----------------------------------------------
        for qb in range(QB):
            qp = psT.tile([128, 1024], f32, tag="pp", name=f"qp{qb}")
            for dq in range(DQC):
                xf = xpool.tile([128, 512], bf, tag="xs", name=f"xfq{qb}_{dq}")
                eng = nc.sync if dq % 2 == 0 else nc.scalar
                eng.dma_start(xf[:], xTq[ts(dq, 128), ts(qb, 512)])
                for cc in range(2):
                    nc.tensor.matmul(qp[:, ts(cc, 512)], wq_r[:, dq, ts(cc, 128)],
                                     xf[:], start=(dq == 0), stop=(dq == DQC - 1))
            for cc in range(2):
                nc.vector.tensor_scalar_add(qT_sb[cc][:, ts(qb, 512)],
                                            qp[:, ts(cc, 512)], bq_v[:, cc:cc + 1])

        # remaining weights (DMA priority after the q-projection inputs)
        wk_r = load_w3("wkr", wk)
        wv_r = load_w3("wvr", wv)
        wo_r = load_w3("wor", wo)               # [128, 2, 1024]
        bk_v = persist.tile([128, 2], f32, tag="bkv", name="bk_v")
        nc.scalar.dma_start(bk_v[:], bk2)
        bv_b = persist.tile([128, CS], f32, tag="bvb", name="bv_b")
        nc.scalar.dma_start(bv_b[:], bvb)
        bv_b3 = bv_b[:].rearrange("p (h c) -> p h c", h=HC)
        ones128 = persist.tile([1, 128], bf, tag="o128", name="ones128")
        nc.vector.memset(ones128[:], 1.0)
        # augmented-V ones column (denominator row), set once
        for kt in range(KT):
            nc.vector.memset(va_sb[kt][:, :, 64:65], 1.0)

        # ---- attention helpers ------------------------------------------
        oPs = {}

        def open_oP(qb, hp, pool, sfx):
            oPs[(qb, hp)] = [
                pool.tile([128, 512], f32, tag=f"oP{qb}{hp}{h}{sfx}",
                          name=f"oP{qb}{hp}{h}{sfx}")
                for h in range(2)
            ]

        def attn_qk(qb, hp, kt):
            kb, kti = kt // 4, kt % 4
            sp = psT.tile([128, 1024], f32, tag="pp", name=f"sp{qb}{hp}{kt}")
            for h in range(2):
                nc.tensor.matmul(
                    sp[:, ts(h, 512)],
                    kT_sb[hp][kb][ts(h, 64), ts(kti, 128)],
                    qT_sb[hp][ts(h, 64), ts(qb, 512)],
                    start=True, stop=True,
                )
            pT = pT_pool.tile([128, 1024], bf, tag="pT", name=f"pT{qb}{hp}{kt}")
            nc.scalar.activation(pT[:], sp[:], mybir.ActivationFunctionType.Exp,
                                 scale=float(SCALE), bias=mb_t[:, kt:kt + 1])
            return pT

        def attn_av(qb, hp, kt, pT):
            oP = oPs[(qb, hp)]
            for h in range(2):
                nc.tensor.matmul(
                    oP[h][0:65, :], va_sb[kt][:, hp * 2 + h, :], pT[:, ts(h, 512)],
                    start=(kt == 0), stop=(kt == KT - 1),
                )

        def attn_norm(qb, hp):
            oP = oPs.pop((qb, hp))
            rb = psT.tile([128, 1024], f32, tag="pp", name=f"rb{qb}{hp}")
            rb_sb = rb_pool.tile([128, 1024], f32, tag="rbs", name=f"rbs{qb}{hp}")
            for h in range(2):
                den = rb_pool.tile([1, 512], f32, tag="den", name=f"den{qb}{hp}{h}")
                nc.vector.tensor_copy(den[:], oP[h][64:65, :])
                rdf = rb_pool.tile([1, 512], f32, tag="rdf", name=f"rdf{qb}{hp}{h}")
                # approx_fast needs an SBUF source (PSUM source returns garbage)
                nc.vector.reciprocal_approx_fast(rdf[:], den[:])
                rd = rb_pool.tile([1, 512], bf, tag="rd", name=f"rd{qb}{hp}{h}")
                nc.vector.tensor_copy(rd[:], rdf[:])
                nc.tensor.matmul(rb[:, ts(h, 512)], ones128[:], rd[:],
                                 start=True, stop=True)
            nc.vector.tensor_copy(rb_sb[:], rb[:])
            for h in range(2):
                nc.vector.tensor_mul(onT_sb[hp][ts(h, 64), ts(qb, 512)],
                                     oP[h][0:64, :], rb_sb[0:64, ts(h, 512)])

        # ---- phase A: K/V projections + attention on hp=0 (both qb) -----
        open_oP(0, 0, psOA, "a")
        open_oP(1, 0, psOA, "a")
        pend_av = []

        def drip():
            if pend_av:
                attn_av(*pend_av.pop(0))

        lag_qk = []
        for kb in range(KB):
            # dense AV burst first: these are runnable (their exps are done
            # or nearly done) and fill the PE while the exp chain frees the
            # projection psum bufs — no boundary stall, HAM stays warm
            while pend_av:
                drip()
            kp = psT.tile([128, 1024], f32, tag="pp", name=f"kp{kb}")
            xks = []
            for dq in range(DQC):
                xf = xpool.tile([128, 512], bf, tag="xs", name=f"xfk{kb}_{dq}")
                eng = nc.sync if dq % 2 == 0 else nc.scalar
                eng.dma_start(xf[:], xTkv[ts(dq, 128), ts(kb, 512)])
                xks.append(xf)
                for cc in range(2):
                    nc.tensor.matmul(kp[:, ts(cc, 512)], wk_r[:, dq, ts(cc, 128)],
                                     xf[:], start=(dq == 0), stop=(dq == DQC - 1))
            for cc in range(2):
                nc.vector.tensor_scalar_add(kT_sb[cc][kb][:], kp[:, ts(cc, 512)],
                                            bk_v[:, cc:cc + 1])

            # one lagged QK before vp (vp then inherits kp's psum slot
            # instead of waiting on an exp), the second after vp's matmuls
            if lag_qk:
                lqb, lhp, lkt = lag_qk.pop(0)
                pend_av.append((lqb, lhp, lkt, attn_qk(lqb, lhp, lkt)))

            vp = psT.tile([128, 1024], f32, tag="pp", name=f"vp{kb}")
            for dq in range(DQC):
                for t in range(4):
                    # start clears has_written for the whole 2KB psum bank, so
                    # only the first matmul touching each bank may set it
                    nc.tensor.matmul(vp[:, ts(t, 256)], xks[dq][:, ts(t, 128)],
                                     wv_r[:, dq, :],
                                     start=(dq == 0 and t % 2 == 0),
                                     stop=(dq == DQC - 1 and t % 2 == 1))
            for (lqb, lhp, lkt) in lag_qk:
                pend_av.append((lqb, lhp, lkt, attn_qk(lqb, lhp, lkt)))
            lag_qk = []
            for t in range(4):
                kt = kb * 4 + t
                src = vp[:, ts(t, 256)].rearrange("p (h c) -> p h c", h=HC)
                nc.vector.tensor_add(va_sb[kt][:, :, 0:64], src, bv_b3)

            for t in range(4):
                kt = kb * 4 + t
                if t == 3:
                    lag_qk = [(qb, 0, kt) for qb in range(QB)]
                    break
                for qb in range(QB):
                    pT = attn_qk(qb, 0, kt)
                    pend_av.append((qb, 0, kt, pT))

        # ---- phase B: attention on hp=1 (both qb ragged) ----------------
        # phase B's QK/exp chain starts right away; the last key-block's
        # hp=0 AV matmuls drain under it, then the phase-A norms run and
        # release the psOA banks for phase B's accumulators.
        for (lqb, lhp, lkt) in lag_qk:
            pend_av.append((lqb, lhp, lkt, attn_qk(lqb, lhp, lkt)))
        lag_qk = []
        for kt in range(3):
            for qb in range(QB):
                pend_av.append((qb, 1, kt, attn_qk(qb, 1, kt)))
            for _ in range(4):
                if pend_av and pend_av[0][1] == 0 and len(pend_av) > 4:
                    drip()
        while pend_av and pend_av[0][1] == 0:
            drip()
        attn_norm(0, 0)
        attn_norm(1, 0)
        psOA_cm.__exit__(None, None, None)
        psOB_cm = tc.tile_pool(name="psOB", bufs=1, space="PSUM")
        psOB = psOB_cm.__enter__()
        open_oP(0, 1, psOB, "b")
        open_oP(1, 1, psOB, "b")
        for kt in range(3, KT):
            for qb in range(QB):
                pend_av.append((qb, 1, kt, attn_qk(qb, 1, kt)))
            while len(pend_av) > 4:
                drip()
        while pend_av:
            drip()

        def oproj(qt):
            # accumulate in the released (0,1) oP banks
            ops = []
            for eb in range(2):
                op = psOB.tile([128, 512], f32, tag=f"oP01{eb}b",
                               name=f"op{qt}_{eb}")
                ops.append(op)
                for cc in range(2):
                    nc.tensor.matmul(op[:, :], onT_sb[cc][:, ts(qt, 128)],
                                     wo_r[:, cc, ts(eb, 512)],
                                     start=(cc == 0), stop=(cc == 1))
            osb = outsb_pool.tile([128, 1024], bf, tag="osb", name=f"osb{qt}")
            for eb in range(2):
                # ACT engine is idle in the tail; DVE runs the norm chains
                nc.scalar.copy(osb[:, ts(eb, 512)], ops[eb][:])
            nc.scalar.dma_start(out[ts(qt, 128), :], osb[:])

        # tail: norms feed the output projection; oproj PE work overlaps
        # the norm DVE chains
        attn_norm(0, 1)
        for qt in range(4):
            oproj(qt)
        attn_norm(1, 1)
        for qt in range(4, 8):
            oproj(qt)

        psOB_cm.__exit__(None, None, None)
        lp.__exit__(None, None, None)


def kernel(x_q, x_kv, pad_mask, Wq, bq, Wk, bk, Wv, bv, Wo, bo):
    global LAST_EXEC_NS
    import ml_dtypes
    bf16 = ml_dtypes.bfloat16

    x_q = np.asarray(x_q, np.float32)
    x_kv = np.asarray(x_kv, np.float32)
    pad_mask = np.asarray(pad_mask)
    Wq, bq = np.asarray(Wq, np.float32), np.asarray(bq, np.float32)
    Wk, bk = np.asarray(Wk, np.float32), np.asarray(bk, np.float32)
    Wv, bv = np.asarray(Wv, np.float32), np.asarray(bv, np.float32)
    Wo, bo = np.asarray(Wo, np.float32), np.asarray(bo, np.float32)

    if "nc" not in _cache:
        _cache["nc"] = _build()
    nc = _cache["nc"]

    xTq_b = [np.ascontiguousarray(x_q[b].T.astype(bf16)) for b in range(B)]
    xTkv_b = [np.ascontiguousarray(x_kv[b].T.astype(bf16)) for b in range(B)]
    mb_b = []
    for b in range(B):
        m = np.where(pad_mask[b], np.float32(MASK_BIAS), np.float32(0.0))
        mb_b.append(np.ascontiguousarray(m.reshape(KT, 128).T.astype(np.float32)))

    in_maps = []
    for c in range(N_CORES):
        b, g = c // 4, c % 4
        hs = g * CS
        in_maps.append({
            "xTq": xTq_b[b],
            "xTkv": xTkv_b[b],
            "wq": np.ascontiguousarray(
                Wq[:, hs:hs + CS].reshape(DQC, 128, CS).transpose(1, 0, 2)
                .astype(bf16)),
            "wk": np.ascontiguousarray(
                Wk[:, hs:hs + CS].reshape(DQC, 128, CS).transpose(1, 0, 2)
                .astype(bf16)),
            "wv": np.ascontiguousarray(
                Wv[:, hs:hs + CS].reshape(DQC, 128, CS).transpose(1, 0, 2)
                .astype(bf16)),
            "wo": np.ascontiguousarray(
                Wo[hs:hs + CS, :].reshape(2, 128, D).transpose(1, 0, 2)
                .astype(bf16)),
            "bq2": np.ascontiguousarray(bq[hs:hs + CS].reshape(2, 128).T),
            "bk2": np.ascontiguousarray(bk[hs:hs + CS].reshape(2, 128).T),
            "bvb": np.ascontiguousarray(
                np.broadcast_to(bv[hs:hs + CS], (128, CS)).astype(np.float32)),
            "mb": mb_b[b],
        })

    res = run_bass_kernel_spmd(nc, in_maps, list(range(N_CORES)), trace=TRACE)
    LAST_EXEC_NS = res.exec_time_ns

    outp = np.zeros((B, N, D), np.float32)
    for c in range(N_CORES):
        outp[c // 4] += res.results[c]["out"].astype(np.float32)
    outp += bo
    return outp


# revision 31
# speedup vs baseline: 1.1887x; 1.0130x over previous
"""Multi-head cross-attention (B=2, N=1024, L=4096, D=1024, H=16) on 8 trn2
NeuronCores — bf16 v2.

Sharding: batch x head-group data/tensor parallel. Core c handles batch
c//4 and heads 4*(c%4) .. 4*(c%4)+3 (weight columns sliced per head group,
Wo row-sliced; partial outputs summed on the host during unsharding).

v2 changes vs the fp32r baseline:
  - all matmul operands bf16 (fp32 PSUM accumulate): fp32 moving operands
    stream at 2 cycles/col on the PE xbus, bf16 at 1 — halves matmul time
    and DMA traffic, and the DMA'd bf16 tiles feed matmuls directly (no
    fp32->fp32r DVE casts).
  - padding mask applied as a per-key additive bias (-60) inside the exp
    activation (bias is a [128,1] per-partition AP), so V needs no keep
    premultiply; the augmented-V ones column provides the denominator.
  - q/k biases folded into the PSUM->SBUF copies (tensor_scalar_add with a
    per-partition bias vector); v bias added during the va build.
  - AV matmuls for key-block kb are dripped into kb+1's projection matmuls
    so the PE never stalls on the exp (ACT) latency.
"""
import sys

sys.path.insert(0, "/opt/trn_rl_repo")

import numpy as np

import concourse.bass as bass
import concourse.tile as tile
from concourse import bacc, mybir
from concourse.bass_utils import run_bass_kernel_spmd

dt = mybir.dt
ts = bass.ts

B, N, L, D = 2, 1024, 4096, 1024
H, DH = 16, 64
HC = 4            # heads per core
CS = HC * DH      # 256 channel slice per core
SCALE = DH ** -0.5
N_CORES = 8
QB, KB = 2, 8     # query blocks of 512, key blocks of 512
DQC = 8           # contraction chunks of 128
KT = 32           # keytiles of 128
MASK_BIAS = -60.0

TRACE = False
LAST_EXEC_NS = None
_cache = {}


def _build():
    nc = bacc.Bacc("TRN2", target_bir_lowering=False, debug=False,
                   num_devices=N_CORES)
    bf = dt.bfloat16

    xTq = nc.dram_tensor("xTq", [D, N], bf, kind="ExternalInput").ap()
    xTkv = nc.dram_tensor("xTkv", [D, L], bf, kind="ExternalInput").ap()
    # weights pre-chunked on the host to [128, chunk, F] so the DMA reads
    # contiguous 4KB-per-partition lines instead of 512B strided rows
    wq = nc.dram_tensor("wq", [128, DQC, CS], bf, kind="ExternalInput").ap()
    wk = nc.dram_tensor("wk", [128, DQC, CS], bf, kind="ExternalInput").ap()
    wv = nc.dram_tensor("wv", [128, DQC, CS], bf, kind="ExternalInput").ap()
    wo = nc.dram_tensor("wo", [128, 2, D], bf, kind="ExternalInput").ap()
    bq2 = nc.dram_tensor("bq2", [128, 2], dt.float32, kind="ExternalInput").ap()
    bk2 = nc.dram_tensor("bk2", [128, 2], dt.float32, kind="ExternalInput").ap()
    bvb = nc.dram_tensor("bvb", [128, CS], dt.float32, kind="ExternalInput").ap()
    mb = nc.dram_tensor("mb", [128, KT], dt.float32, kind="ExternalInput").ap()
    out = nc.dram_tensor("out", [N, D], bf, kind="ExternalOutput").ap()

    with tile.TileContext(nc) as tc:
        _emit(nc, tc, xTq, xTkv, wq, wk, wv, wo, bq2, bk2, bvb, mb, out)
    nc.compile()
    return nc


def _emit(nc, tc, xTq, xTkv, wq, wk, wv, wo, bq2, bk2, bvb, mb, out):
    import contextlib

    bf = dt.bfloat16
    f32 = dt.float32
    ctx = contextlib.ExitStack()
    with ctx:
        persist = ctx.enter_context(tc.tile_pool(name="persist", bufs=1))
        xpool = ctx.enter_context(tc.tile_pool(name="xs", bufs=12))
        pT_pool = ctx.enter_context(tc.tile_pool(name="pT", bufs=18))
        rb_pool = ctx.enter_context(tc.tile_pool(name="rbs", bufs=2))
        outsb_pool = ctx.enter_context(tc.tile_pool(name="outsb", bufs=2))
        psT = ctx.enter_context(tc.tile_pool(name="psT", bufs=2, space="PSUM"))
        psOA_cm = tc.tile_pool(name="psOA", bufs=1, space="PSUM")
        psOA = psOA_cm.__enter__()
        lp = nc.allow_low_precision(reason="bf16 attention internals")
        lp.__enter__()

        def load_w3(name, src):
            # src: DRAM [128, d0, F] bf16 (host pre-chunked, contiguous).
            # Weights ride the ACT hardware DMA queue so they stream in
            # parallel with the x DMAs on the SP queue.
            r = persist.tile(list(src.shape), bf, tag=name, name=name)
            nc.scalar.dma_start(r[:], src)
            return r

        # ---- weights needed for the Q projection ------------------------
        wq_r = load_w3("wqr", wq)               # [128, 8, 256]
        bq_v = persist.tile([128, 2], f32, tag="bqv", name="bq_v")
        nc.scalar.dma_start(bq_v[:], bq2)
        mb_t = persist.tile([128, KT], f32, tag="mbt", name="mb_t")
        nc.scalar.dma_start(mb_t[:], mb)

        # ---- persistent activation tiles --------------------------------
        qT_sb = [persist.tile([128, N], bf, tag=f"qT{cc}", name=f"qT{cc}")
                 for cc in range(2)]
        kT_sb = [[persist.tile([128, 512], bf, tag=f"kT{cc}_{kb}",
                               name=f"kT{cc}_{kb}") for kb in range(KB)]
                 for cc in range(2)]
        va_sb = [persist.tile([128, HC, 65], bf, tag=f"va{kt}",
                              name=f"va{kt}") for kt in range(KT)]
        onT_sb = [persist.tile([128, N], bf, tag=f"onT{cc}",
                               name=f"onT{cc}") for cc in range(2)]

        # ---- Q projection ----------------------------------------------
        for qb in range(QB):
            qp = psT.tile([128, 1024], f32, tag="pp", name=f"qp{qb}")
            for dq in range(DQC):
                xf = xpool.tile([128, 512], bf, tag="xs", name=f"xfq{qb}_{dq}")
                eng = nc.sync if dq % 2 == 0 else nc.scalar
                eng.dma_start(xf[:], xTq[ts(dq, 128), ts(qb, 512)])
                for cc in range(2):
                    nc.tensor.matmul(qp[:, ts(cc, 512)], wq_r[:, dq, ts(cc, 128)],
                                     xf[:], start=(dq == 0), stop=(dq == DQC - 1))
            for cc in range(2):
                nc.vector.tensor_scalar_add(qT_sb[cc][:, ts(qb, 512)],
                                            qp[:, ts(cc, 512)], bq_v[:, cc:cc + 1])

        # remaining weights (DMA priority after the q-projection inputs)
        wk_r = load_w3("wkr", wk)
        wv_r = load_w3("wvr", wv)
        wo_r = load_w3("wor", wo)               # [128, 2, 1024]
        bk_v = persist.tile([128, 2], f32, tag="bkv", name="bk_v")
        nc.scalar.dma_start(bk_v[:], bk2)
        bv_b = persist.tile([128, CS], f32, tag="bvb", name="bv_b")
        nc.scalar.dma_start(bv_b[:], bvb)
        bv_b3 = bv_b[:].rearrange("p (h c) -> p h c", h=HC)
        ones128 = persist.tile([1, 128], bf, tag="o128", name="ones128")
        nc.vector.memset(ones128[:], 1.0)
        # augmented-V ones column (denominator row), set once
        for kt in range(KT):
            nc.vector.memset(va_sb[kt][:, :, 64:65], 1.0)

        # ---- attention helpers ------------------------------------------
        oPs = {}

        def open_oP(qb, hp, pool, sfx):
            oPs[(qb, hp)] = [
                pool.tile([128, 512], f32, tag=f"oP{qb}{hp}{h}{sfx}",
                          name=f"oP{qb}{hp}{h}{sfx}")
                for h in range(2)
            ]

        def attn_qk(qb, hp, kt):
            kb, kti = kt // 4, kt % 4
            sp = psT.tile([128, 1024], f32, tag="pp", name=f"sp{qb}{hp}{kt}")
            for h in range(2):
                nc.tensor.matmul(
                    sp[:, ts(h, 512)],
                    kT_sb[hp][kb][ts(h, 64), ts(kti, 128)],
                    qT_sb[hp][ts(h, 64), ts(qb, 512)],
                    start=True, stop=True,
                )
            pT = pT_pool.tile([128, 1024], bf, tag="pT", name=f"pT{qb}{hp}{kt}")
            nc.scalar.activation(pT[:], sp[:], mybir.ActivationFunctionType.Exp,
                                 scale=float(SCALE), bias=mb_t[:, kt:kt + 1])
            return pT

        def attn_av(qb, hp, kt, pT):
            oP = oPs[(qb, hp)]
            for h in range(2):
                nc.tensor.matmul(
                    oP[h][0:65, :], va_sb[kt][:, hp * 2 + h, :], pT[:, ts(h, 512)],
                    start=(kt == 0), stop=(kt == KT - 1),
                )

        def attn_norm(qb, hp):
            oP = oPs.pop((qb, hp))
            rb = psT.tile([128, 1024], f32, tag="pp", name=f"rb{qb}{hp}")
            rb_sb = rb_pool.tile([128, 1024], f32, tag="rbs", name=f"rbs{qb}{hp}")
            for h in range(2):
                den = rb_pool.tile([1, 512], f32, tag="den", name=f"den{qb}{hp}{h}")
                nc.vector.tensor_copy(den[:], oP[h][64:65, :])
                rdf = rb_pool.tile([1, 512], f32, tag="rdf", name=f"rdf{qb}{hp}{h}")
                # approx_fast needs an SBUF source (PSUM source returns garbage)
                nc.vector.reciprocal_approx_fast(rdf[:], den[:])
                rd = rb_pool.tile([1, 512], bf, tag="rd", name=f"rd{qb}{hp}{h}")
                nc.vector.tensor_copy(rd[:], rdf[:])
                nc.tensor.matmul(rb[:, ts(h, 512)], ones128[:], rd[:],
                                 start=True, stop=True)
            nc.vector.tensor_copy(rb_sb[:], rb[:])
            for h in range(2):
                nc.vector.tensor_mul(onT_sb[hp][ts(h, 64), ts(qb, 512)],
                                     oP[h][0:64, :], rb_sb[0:64, ts(h, 512)])

        # ---- phase A: K/V projections + attention on hp=0 (both qb) -----
        open_oP(0, 0, psOA, "a")
        open_oP(1, 0, psOA, "a")
        pend_av = []

        def drip():
            if pend_av:
                attn_av(*pend_av.pop(0))

        lag_qk = []
        for kb in range(KB):
            # dense AV burst first: these are runnable (their exps are done
            # or nearly done) and fill the PE while the exp chain frees the
            # projection psum bufs — no boundary stall, HAM stays warm
            while pend_av:
                drip()
            kp = psT.tile([128, 1024], f32, tag="pp", name=f"kp{kb}")
            xks = []
            for dq in range(DQC):
                xf = xpool.tile([128, 512], bf, tag="xs", name=f"xfk{kb}_{dq}")
                eng = nc.sync if dq % 2 == 0 else nc.scalar
                eng.dma_start(xf[:], xTkv[ts(dq, 128), ts(kb, 512)])
                xks.append(xf)
                for cc in range(2):
                    nc.tensor.matmul(kp[:, ts(cc, 512)], wk_r[:, dq, ts(cc, 128)],
                                     xf[:], start=(dq == 0), stop=(dq == DQC - 1))
            for cc in range(2):
                nc.vector.tensor_scalar_add(kT_sb[cc][kb][:], kp[:, ts(cc, 512)],
                                            bk_v[:, cc:cc + 1])

            # lagged QK of the previous key-block's last keytile: its exp
            # runs while this block's V projection streams on the PE
            for (lqb, lhp, lkt) in lag_qk:
                pend_av.append((lqb, lhp, lkt, attn_qk(lqb, lhp, lkt)))
            lag_qk = []

            vp = psT.tile([128, 1024], f32, tag="pp", name=f"vp{kb}")
            for dq in range(DQC):
                for t in range(4):
                    # start clears has_written for the whole 2KB psum bank, so
                    # only the first matmul touching each bank may set it
                    nc.tensor.matmul(vp[:, ts(t, 256)], xks[dq][:, ts(t, 128)],
                                     wv_r[:, dq, :],
                                     start=(dq == 0 and t % 2 == 0),
                                     stop=(dq == DQC - 1 and t % 2 == 1))
            for t in range(4):
                kt = kb * 4 + t
                src = vp[:, ts(t, 256)].rearrange("p (h c) -> p h c", h=HC)
                nc.vector.tensor_add(va_sb[kt][:, :, 0:64], src, bv_b3)

            for t in range(4):
                kt = kb * 4 + t
                if t == 3:
                    lag_qk = [(qb, 0, kt) for qb in range(QB)]
                    break
                for qb in range(QB):
                    pT = attn_qk(qb, 0, kt)
                    pend_av.append((qb, 0, kt, pT))

        # ---- phase B: attention on hp=1 (both qb ragged) ----------------
        # phase B's QK/exp chain starts right away; the last key-block's
        # hp=0 AV matmuls drain under it, then the phase-A norms run and
        # release the psOA banks for phase B's accumulators.
        for (lqb, lhp, lkt) in lag_qk:
            pend_av.append((lqb, lhp, lkt, attn_qk(lqb, lhp, lkt)))
        lag_qk = []
        for kt in range(3):
            for qb in range(QB):
                pend_av.append((qb, 1, kt, attn_qk(qb, 1, kt)))
            for _ in range(4):
                if pend_av and pend_av[0][1] == 0 and len(pend_av) > 4:
                    drip()
        while pend_av and pend_av[0][1] == 0:
            drip()
        attn_norm(0, 0)
        attn_norm(1, 0)
        psOA_cm.__exit__(None, None, None)
        psOB_cm = tc.tile_pool(name="psOB", bufs=1, space="PSUM")
        psOB = psOB_cm.__enter__()
        open_oP(0, 1, psOB, "b")
        open_oP(1, 1, psOB, "b")
        for kt in range(3, KT):
            for qb in range(QB):
                pend_av.append((qb, 1, kt, attn_qk(qb, 1, kt)))
            while len(pend_av) > 4:
                drip()
        while pend_av:
            drip()

        def oproj(qt):
            # accumulate in the released (0,1) oP banks
            ops = []
            for eb in range(2):
                op = psOB.tile([128, 512], f32, tag=f"oP01{eb}b",
                               name=f"op{qt}_{eb}")
                ops.append(op)
                for cc in range(2):
                    nc.tensor.matmul(op[:, :], onT_sb[cc][:, ts(qt, 128)],
                                     wo_r[:, cc, ts(eb, 512)],
                                     start=(cc == 0), stop=(cc == 1))
            osb = outsb_pool.tile([128, 1024], bf, tag="osb", name=f"osb{qt}")
            # split the two casts across ACT and DVE so they run in
            # parallel; alternate the out DMAs across both hardware queues
            nc.scalar.copy(osb[:, ts(0, 512)], ops[0][:])
            nc.vector.tensor_copy(osb[:, ts(1, 512)], ops[1][:])
            eng = nc.sync if qt % 2 == 0 else nc.scalar
            eng.dma_start(out[ts(qt, 128), :], osb[:])

        # tail: norms feed the output projection; oproj PE work overlaps
        # the norm DVE chains
        attn_norm(0, 1)
        for qt in range(4):
            oproj(qt)
        attn_norm(1, 1)
        for qt in range(4, 8):
            oproj(qt)

        psOB_cm.__exit__(None, None, None)
        lp.__exit__(None, None, None)


def kernel(x_q, x_kv, pad_mask, Wq, bq, Wk, bk, Wv, bv, Wo, bo):
    global LAST_EXEC_NS
    import ml_dtypes
    bf16 = ml_dtypes.bfloat16

    x_q = np.asarray(x_q, np.float32)
    x_kv = np.asarray(x_kv, np.float32)
    pad_mask = np.asarray(pad_mask)
    Wq, bq = np.asarray(Wq, np.float32), np.asarray(bq, np.float32)
    Wk, bk = np.asarray(Wk, np.float32), np.asarray(bk, np.float32)
    Wv, bv = np.asarray(Wv, np.float32), np.asarray(bv, np.float32)
    Wo, bo = np.asarray(Wo, np.float32), np.asarray(bo, np.float32)

    if "nc" not in _cache:
        _cache["nc"] = _build()
    nc = _cache["nc"]

    xTq_b = [np.ascontiguousarray(x_q[b].T.astype(bf16)) for b in range(B)]
    xTkv_b = [np.ascontiguousarray(x_kv[b].T.astype(bf16)) for b in range(B)]
    mb_b = []
    for b in range(B):
        m = np.where(pad_mask[b], np.float32(MASK_BIAS), np.float32(0.0))
        mb_b.append(np.ascontiguousarray(m.reshape(KT, 128).T.astype(np.float32)))

    in_maps = []
    for c in range(N_CORES):
        b, g = c // 4, c % 4
        hs = g * CS
        in_maps.append({
            "xTq": xTq_b[b],
            "xTkv": xTkv_b[b],
            "wq": np.ascontiguousarray(
                Wq[:, hs:hs + CS].reshape(DQC, 128, CS).transpose(1, 0, 2)
                .astype(bf16)),
            "wk": np.ascontiguousarray(
                Wk[:, hs:hs + CS].reshape(DQC, 128, CS).transpose(1, 0, 2)
                .astype(bf16)),
            "wv": np.ascontiguousarray(
                Wv[:, hs:hs + CS].reshape(DQC, 128, CS).transpose(1, 0, 2)
                .astype(bf16)),
            "wo": np.ascontiguousarray(
                Wo[hs:hs + CS, :].reshape(2, 128, D).transpose(1, 0, 2)
                .astype(bf16)),
            "bq2": np.ascontiguousarray(bq[hs:hs + CS].reshape(2, 128).T),
            "bk2": np.ascontiguousarray(bk[hs:hs + CS].reshape(2, 128).T),
            "bvb": np.ascontiguousarray(
                np.broadcast_to(bv[hs:hs + CS], (128, CS)).astype(np.float32)),
            "mb": mb_b[b],
        })

    res = run_bass_kernel_spmd(nc, in_maps, list(range(N_CORES)), trace=TRACE)
    LAST_EXEC_NS = res.exec_time_ns

    outp = np.zeros((B, N, D), np.float32)
    for c in range(N_CORES):
        outp[c // 4] += res.results[c]["out"].astype(np.float32)
    outp += bo
    return outp


# revision 32
# speedup vs baseline: 1.2106x; 1.0184x over previous
"""Multi-head cross-attention (B=2, N=1024, L=4096, D=1024, H=16) on 8 trn2
NeuronCores — bf16 v2.

Sharding: batch x head-group data/tensor parallel. Core c handles batch
c//4 and heads 4*(c%4) .. 4*(c%4)+3 (weight columns sliced per head group,
Wo row-sliced; partial outputs summed on the host during unsharding).

v2 changes vs the fp32r baseline:
  - all matmul operands bf16 (fp32 PSUM accumulate): fp32 moving operands
    stream at 2 cycles/col on the PE xbus, bf16 at 1 — halves matmul time
    and DMA traffic, and the DMA'd bf16 tiles feed matmuls directly (no
    fp32->fp32r DVE casts).
  - padding mask applied as a per-key additive bias (-60) inside the exp
    activation (bias is a [128,1] per-partition AP), so V needs no keep
    premultiply; the augmented-V ones column provides the denominator.
  - q/k biases folded into the PSUM->SBUF copies (tensor_scalar_add with a
    per-partition bias vector); v bias added during the va build.
  - AV matmuls for key-block kb are dripped into kb+1's projection matmuls
    so the PE never stalls on the exp (ACT) latency.
"""
import sys

sys.path.insert(0, "/opt/trn_rl_repo")

import numpy as np

import concourse.bass as bass
import concourse.tile as tile
from concourse import bacc, mybir
from concourse.bass_utils import run_bass_kernel_spmd

dt = mybir.dt
ts = bass.ts

B, N, L, D = 2, 1024, 4096, 1024
H, DH = 16, 64
HC = 4            # heads per core
CS = HC * DH      # 256 channel slice per core
SCALE = DH ** -0.5
N_CORES = 8
QB, KB = 2, 8     # query blocks of 512, key blocks of 512
DQC = 8           # contraction chunks of 128
KT = 32           # keytiles of 128
MASK_BIAS = -60.0

TRACE = False
LAST_EXEC_NS = None
_cache = {}


def _build():
    nc = bacc.Bacc("TRN2", target_bir_lowering=False, debug=False,
                   num_devices=N_CORES)
    bf = dt.bfloat16

    xTq = nc.dram_tensor("xTq", [D, N], bf, kind="ExternalInput").ap()
    xTkv = nc.dram_tensor("xTkv", [D, L], bf, kind="ExternalInput").ap()
    # weights pre-chunked on the host to [128, chunk, F] so the DMA reads
    # contiguous 4KB-per-partition lines instead of 512B strided rows
    wq = nc.dram_tensor("wq", [128, DQC, CS], bf, kind="ExternalInput").ap()
    wk = nc.dram_tensor("wk", [128, DQC, CS], bf, kind="ExternalInput").ap()
    wv = nc.dram_tensor("wv", [128, DQC, CS], bf, kind="ExternalInput").ap()
    wo = nc.dram_tensor("wo", [128, 2, D], bf, kind="ExternalInput").ap()
    bq2 = nc.dram_tensor("bq2", [128, 2], dt.float32, kind="ExternalInput").ap()
    bk2 = nc.dram_tensor("bk2", [128, 2], dt.float32, kind="ExternalInput").ap()
    bvb = nc.dram_tensor("bvb", [128, CS], dt.float32, kind="ExternalInput").ap()
    mb = nc.dram_tensor("mb", [128, KT], dt.float32, kind="ExternalInput").ap()
    out = nc.dram_tensor("out", [N, D], bf, kind="ExternalOutput").ap()

    with tile.TileContext(nc) as tc:
        _emit(nc, tc, xTq, xTkv, wq, wk, wv, wo, bq2, bk2, bvb, mb, out)
    nc.compile()
    return nc


def _emit(nc, tc, xTq, xTkv, wq, wk, wv, wo, bq2, bk2, bvb, mb, out):
    import contextlib

    bf = dt.bfloat16
    f32 = dt.float32
    ctx = contextlib.ExitStack()
    with ctx:
        persist = ctx.enter_context(tc.tile_pool(name="persist", bufs=1))
        xpool = ctx.enter_context(tc.tile_pool(name="xs", bufs=12))
        pT_pool = ctx.enter_context(tc.tile_pool(name="pT", bufs=18))
        rb_pool = ctx.enter_context(tc.tile_pool(name="rbs", bufs=2))
        outsb_pool = ctx.enter_context(tc.tile_pool(name="outsb", bufs=2))
        psT = ctx.enter_context(tc.tile_pool(name="psT", bufs=2, space="PSUM"))
        psOA_cm = tc.tile_pool(name="psOA", bufs=1, space="PSUM")
        psOA = psOA_cm.__enter__()
        lp = nc.allow_low_precision(reason="bf16 attention internals")
        lp.__enter__()

        def load_w3(name, src):
            # src: DRAM [128, d0, F] bf16 (host pre-chunked, contiguous).
            # Weights ride the ACT hardware DMA queue so they stream in
            # parallel with the x DMAs on the SP queue.
            r = persist.tile(list(src.shape), bf, tag=name, name=name)
            nc.scalar.dma_start(r[:], src)
            return r

        # ---- weights needed for the Q projection ------------------------
        wq_r = load_w3("wqr", wq)               # [128, 8, 256]
        bq_v = persist.tile([128, 2], f32, tag="bqv", name="bq_v")
        nc.scalar.dma_start(bq_v[:], bq2)
        mb_t = persist.tile([128, KT], f32, tag="mbt", name="mb_t")
        nc.scalar.dma_start(mb_t[:], mb)

        # ---- persistent activation tiles --------------------------------
        qT_sb = [persist.tile([128, N], bf, tag=f"qT{cc}", name=f"qT{cc}")
                 for cc in range(2)]
        kT_sb = [[persist.tile([128, 512], bf, tag=f"kT{cc}_{kb}",
                               name=f"kT{cc}_{kb}") for kb in range(KB)]
                 for cc in range(2)]
        va_sb = [persist.tile([128, HC, 65], bf, tag=f"va{kt}",
                              name=f"va{kt}") for kt in range(KT)]
        onT_sb = [persist.tile([128, N], bf, tag=f"onT{cc}",
                               name=f"onT{cc}") for cc in range(2)]

        # ---- Q projection ----------------------------------------------
        for qb in range(QB):
            qp = psT.tile([128, 1024], f32, tag="pp", name=f"qp{qb}")
            for dq in range(DQC):
                xf = xpool.tile([128, 512], bf, tag="xs", name=f"xfq{qb}_{dq}")
                eng = nc.sync if dq % 2 == 0 else nc.scalar
                eng.dma_start(xf[:], xTq[ts(dq, 128), ts(qb, 512)])
                for cc in range(2):
                    nc.tensor.matmul(qp[:, ts(cc, 512)], wq_r[:, dq, ts(cc, 128)],
                                     xf[:], start=(dq == 0), stop=(dq == DQC - 1))
            for cc in range(2):
                nc.vector.tensor_scalar_add(qT_sb[cc][:, ts(qb, 512)],
                                            qp[:, ts(cc, 512)], bq_v[:, cc:cc + 1])

        # remaining weights (DMA priority after the q-projection inputs)
        wk_r = load_w3("wkr", wk)
        wv_r = load_w3("wvr", wv)
        wo_r = load_w3("wor", wo)               # [128, 2, 1024]
        bk_v = persist.tile([128, 2], f32, tag="bkv", name="bk_v")
        nc.scalar.dma_start(bk_v[:], bk2)
        bv_b = persist.tile([128, CS], f32, tag="bvb", name="bv_b")
        nc.scalar.dma_start(bv_b[:], bvb)
        bv_b3 = bv_b[:].rearrange("p (h c) -> p h c", h=HC)
        ones128 = persist.tile([1, 128], bf, tag="o128", name="ones128")
        nc.vector.memset(ones128[:], 1.0)
        # augmented-V ones column (denominator row), set once
        for kt in range(KT):
            nc.vector.memset(va_sb[kt][:, :, 64:65], 1.0)

        # ---- attention helpers ------------------------------------------
        oPs = {}

        def open_oP(qb, hp, pool, sfx):
            oPs[(qb, hp)] = [
                pool.tile([128, 512], f32, tag=f"oP{qb}{hp}{h}{sfx}",
                          name=f"oP{qb}{hp}{h}{sfx}")
                for h in range(2)
            ]

        def attn_qk(qb, hp, kt):
            kb, kti = kt // 4, kt % 4
            sp = psT.tile([128, 1024], f32, tag="pp", name=f"sp{qb}{hp}{kt}")
            for h in range(2):
                nc.tensor.matmul(
                    sp[:, ts(h, 512)],
                    kT_sb[hp][kb][ts(h, 64), ts(kti, 128)],
                    qT_sb[hp][ts(h, 64), ts(qb, 512)],
                    start=True, stop=True,
                )
            pT = pT_pool.tile([128, 1024], bf, tag="pT", name=f"pT{qb}{hp}{kt}")
            nc.scalar.activation(pT[:], sp[:], mybir.ActivationFunctionType.Exp,
                                 scale=float(SCALE), bias=mb_t[:, kt:kt + 1])
            return pT

        def attn_av(qb, hp, kt, pT):
            oP = oPs[(qb, hp)]
            for h in range(2):
                nc.tensor.matmul(
                    oP[h][0:65, :], va_sb[kt][:, hp * 2 + h, :], pT[:, ts(h, 512)],
                    start=(kt == 0), stop=(kt == KT - 1),
                )

        def attn_norm(qb, hp):
            oP = oPs.pop((qb, hp))
            rb = psT.tile([128, 1024], f32, tag="pp", name=f"rb{qb}{hp}")
            rb_sb = rb_pool.tile([128, 1024], f32, tag="rbs", name=f"rbs{qb}{hp}")
            for h in range(2):
                den = rb_pool.tile([1, 512], f32, tag="den", name=f"den{qb}{hp}{h}")
                nc.vector.tensor_copy(den[:], oP[h][64:65, :])
                rdf = rb_pool.tile([1, 512], f32, tag="rdf", name=f"rdf{qb}{hp}{h}")
                # approx_fast needs an SBUF source (PSUM source returns garbage)
                nc.vector.reciprocal_approx_fast(rdf[:], den[:])
                rd = rb_pool.tile([1, 512], bf, tag="rd", name=f"rd{qb}{hp}{h}")
                nc.vector.tensor_copy(rd[:], rdf[:])
                nc.tensor.matmul(rb[:, ts(h, 512)], ones128[:], rd[:],
                                 start=True, stop=True)
            nc.vector.tensor_copy(rb_sb[:], rb[:])
            for h in range(2):
                nc.vector.tensor_mul(onT_sb[hp][ts(h, 64), ts(qb, 512)],
                                     oP[h][0:64, :], rb_sb[0:64, ts(h, 512)])

        # ---- phase A: K/V projections + attention on hp=0 (both qb) -----
        open_oP(0, 0, psOA, "a")
        open_oP(1, 0, psOA, "a")
        pend_av = []

        def drip():
            if pend_av:
                attn_av(*pend_av.pop(0))

        lag_qk = []
        for kb in range(KB):
            # dense AV burst first: these are runnable (their exps are done
            # or nearly done) and fill the PE while the exp chain frees the
            # projection psum bufs — no boundary stall, HAM stays warm
            while pend_av:
                drip()
            kp = psT.tile([128, 1024], f32, tag="pp", name=f"kp{kb}")
            xks = []
            for dq in range(DQC):
                xf = xpool.tile([128, 512], bf, tag="xs", name=f"xfk{kb}_{dq}")
                eng = nc.sync if dq % 2 == 0 else nc.scalar
                eng.dma_start(xf[:], xTkv[ts(dq, 128), ts(kb, 512)])
                xks.append(xf)
                for cc in range(2):
                    nc.tensor.matmul(kp[:, ts(cc, 512)], wk_r[:, dq, ts(cc, 128)],
                                     xf[:], start=(dq == 0), stop=(dq == DQC - 1))
            for cc in range(2):
                nc.vector.tensor_scalar_add(kT_sb[cc][kb][:], kp[:, ts(cc, 512)],
                                            bk_v[:, cc:cc + 1])

            # lagged QK of the previous key-block's last keytile: its exp
            # runs while this block's V projection streams on the PE
            for (lqb, lhp, lkt) in lag_qk:
                pend_av.append((lqb, lhp, lkt, attn_qk(lqb, lhp, lkt)))
            lag_qk = []

            vp = psT.tile([128, 1024], f32, tag="pp", name=f"vp{kb}")
            for dq in range(DQC):
                for t in range(4):
                    # start clears has_written for the whole 2KB psum bank, so
                    # only the first matmul touching each bank may set it
                    nc.tensor.matmul(vp[:, ts(t, 256)], xks[dq][:, ts(t, 128)],
                                     wv_r[:, dq, :],
                                     start=(dq == 0 and t % 2 == 0),
                                     stop=(dq == DQC - 1 and t % 2 == 1))
            for t in range(4):
                kt = kb * 4 + t
                src = vp[:, ts(t, 256)].rearrange("p (h c) -> p h c", h=HC)
                nc.vector.tensor_add(va_sb[kt][:, :, 0:64], src, bv_b3)

            for t in range(4):
                kt = kb * 4 + t
                if t == 3:
                    lag_qk = [(qb, 0, kt) for qb in range(QB)]
                    break
                for qb in range(QB):
                    pT = attn_qk(qb, 0, kt)
                    pend_av.append((qb, 0, kt, pT))

        # ---- phase B: attention on hp=1 (both qb ragged) ----------------
        # phase B's QK/exp chain starts right away; the last key-block's
        # hp=0 AV matmuls drain under it, then the phase-A norms run and
        # release the psOA banks for phase B's accumulators.
        for (lqb, lhp, lkt) in lag_qk:
            pend_av.append((lqb, lhp, lkt, attn_qk(lqb, lhp, lkt)))
        lag_qk = []
        for kt in range(3):
            for qb in range(QB):
                pend_av.append((qb, 1, kt, attn_qk(qb, 1, kt)))
            for _ in range(4):
                if pend_av and pend_av[0][1] == 0 and len(pend_av) > 4:
                    drip()
        while pend_av and pend_av[0][1] == 0:
            drip()
        attn_norm(0, 0)
        attn_norm(1, 0)
        psOA_cm.__exit__(None, None, None)
        psOB_cm = tc.tile_pool(name="psOB", bufs=1, space="PSUM")
        psOB = psOB_cm.__enter__()
        open_oP(0, 1, psOB, "b")
        open_oP(1, 1, psOB, "b")
        for kt in range(3, KT):
            for qb in range(QB):
                pend_av.append((qb, 1, kt, attn_qk(qb, 1, kt)))
            while len(pend_av) > 4:
                drip()
        while pend_av:
            drip()

        def oproj(qt):
            # accumulate in the released (0,1) oP banks
            ops = []
            for eb in range(2):
                op = psOB.tile([128, 512], f32, tag=f"oP01{eb}b",
                               name=f"op{qt}_{eb}")
                ops.append(op)
                for cc in range(2):
                    nc.tensor.matmul(op[:, :], onT_sb[cc][:, ts(qt, 128)],
                                     wo_r[:, cc, ts(eb, 512)],
                                     start=(cc == 0), stop=(cc == 1))
            osb = outsb_pool.tile([128, 1024], bf, tag="osb", name=f"osb{qt}")
            for eb in range(2):
                # ACT engine is idle in the tail; DVE runs the norm chains
                nc.scalar.copy(osb[:, ts(eb, 512)], ops[eb][:])
            nc.scalar.dma_start(out[ts(qt, 128), :], osb[:])

        # tail: norms feed the output projection; oproj PE work overlaps
        # the norm DVE chains
        attn_norm(0, 1)
        for qt in range(4):
            oproj(qt)
        attn_norm(1, 1)
        for qt in range(4, 8):
            oproj(qt)

        psOB_cm.__exit__(None, None, None)
        lp.__exit__(None, None, None)


def kernel(x_q, x_kv, pad_mask, Wq, bq, Wk, bk, Wv, bv, Wo, bo):
    global LAST_EXEC_NS
    import ml_dtypes
    bf16 = ml_dtypes.bfloat16

    x_q = np.asarray(x_q, np.float32)
    x_kv = np.asarray(x_kv, np.float32)
    pad_mask = np.asarray(pad_mask)
    Wq, bq = np.asarray(Wq, np.float32), np.asarray(bq, np.float32)
    Wk, bk = np.asarray(Wk, np.float32), np.asarray(bk, np.float32)
    Wv, bv = np.asarray(Wv, np.float32), np.asarray(bv, np.float32)
    Wo, bo = np.asarray(Wo, np.float32), np.asarray(bo, np.float32)

    if "nc" not in _cache:
        _cache["nc"] = _build()
    nc = _cache["nc"]

    xTq_b = [np.ascontiguousarray(x_q[b].T.astype(bf16)) for b in range(B)]
    xTkv_b = [np.ascontiguousarray(x_kv[b].T.astype(bf16)) for b in range(B)]
    mb_b = []
    for b in range(B):
        m = np.where(pad_mask[b], np.float32(MASK_BIAS), np.float32(0.0))
        mb_b.append(np.ascontiguousarray(m.reshape(KT, 128).T.astype(np.float32)))

    in_maps = []
    for c in range(N_CORES):
        b, g = c // 4, c % 4
        hs = g * CS
        in_maps.append({
            "xTq": xTq_b[b],
            "xTkv": xTkv_b[b],
            "wq": np.ascontiguousarray(
                Wq[:, hs:hs + CS].reshape(DQC, 128, CS).transpose(1, 0, 2)
                .astype(bf16)),
            "wk": np.ascontiguousarray(
                Wk[:, hs:hs + CS].reshape(DQC, 128, CS).transpose(1, 0, 2)
                .astype(bf16)),
            "wv": np.ascontiguousarray(
                Wv[:, hs:hs + CS].reshape(DQC, 128, CS).transpose(1, 0, 2)
                .astype(bf16)),
            "wo": np.ascontiguousarray(
                Wo[hs:hs + CS, :].reshape(2, 128, D).transpose(1, 0, 2)
                .astype(bf16)),
            "bq2": np.ascontiguousarray(bq[hs:hs + CS].reshape(2, 128).T),
            "bk2": np.ascontiguousarray(bk[hs:hs + CS].reshape(2, 128).T),
            "bvb": np.ascontiguousarray(
                np.broadcast_to(bv[hs:hs + CS], (128, CS)).astype(np.float32)),
            "mb": mb_b[b],
        })

    res = run_bass_kernel_spmd(nc, in_maps, list(range(N_CORES)), trace=TRACE)
    LAST_EXEC_NS = res.exec_time_ns

    outp = np.zeros((B, N, D), np.float32)
    for c in range(N_CORES):
        outp[c // 4] += res.results[c]["out"].astype(np.float32)
    outp += bo
    return outp
